# revision 45
# baseline (speedup 1.0000x reference)
"""Causal multi-head attention block (B=2, T=2048, D=1024, H=16) on 8 TRN2 cores.

Sharding: tensor-parallel over heads — each core owns 2 heads (128 cols of
w_attn's q/k/v blocks, 128 rows of w_proj) and produces a partial output
[B, T, D]; the host sums the 8 partials and adds the bias terms.

Per-core kernel (all matmuls in float32r = full PE rate, fp32 accumulation):
  phase 1 (QKV):  qT,kT [128f, B*T] = w^T @ x^T   (w stationary, xT moving)
                  v     [B*T, 128]  = x @ w_v     (xT tiles stationary)
                  v stored per (t-tile, head) as [128, 65] with a ones column
                  FIRST — the ones column makes the AV matmul also produce the
                  softmax denominator.
  phase 2 (attn): per (b, head, q-chunk of 512):
                  sT [128k, 512q] = k @ qT (scores, transposed layout; causal
                  block-skipping: only k-tiles <= q-chunk get computed)
                  probs = exp(sT) (ACT), straddle blocks get multiplicative
                  {0,1} mask strips (DVE)
                  avT [65, 512] += [ones|v]^T @ probs  (row 0 = sum of exp)
                  aT = avT[1:65] * (1/avT[0]) broadcast  -> a_sb (heads stacked)
  phase 3 (proj): out [128t, 512e] = a_sb^T(128=2 heads) @ w_proj rows
"""
import numpy as np

import concourse.bass as bass
import concourse.mybir as mybir
import concourse.tile as tile
from concourse import bacc
from concourse.bass import ts, ds
from concourse.bass_utils import run_bass_kernel_spmd

F32 = mybir.dt.float32
F32R = mybir.dt.float32r
BF16 = mybir.dt.bfloat16

B, T, D = 2, 2048, 1024
H = 16
HD = D // H          # 64
N_CORES = 8
HPC = H // N_CORES   # heads per core = 2
CW = HPC * HD        # per-core head width = 128
TCH = 512            # q/t chunk width
NTCH = (B * T) // TCH   # 8 t-chunks over flattened (b, t)
NKT = T // 128       # 16 k-tiles per batch
NQC = T // TCH       # 4 q-chunks per batch


def build_program(reps: int = 1, phases: str = "123"):
    """Build the per-core Bass program (same program on all 8 cores).

    reps>1 wraps the body in a dynamic loop for wall-clock timing runs.
    """
    nc = bacc.Bacc("TRN2", target_bir_lowering=False, debug=False,
                   num_devices=N_CORES)

    xT = nc.dram_tensor("xT", [B, D, T], F32R, kind="ExternalInput")
    wq = nc.dram_tensor("wq", [D, CW], F32R, kind="ExternalInput")
    wk = nc.dram_tensor("wk", [D, CW], F32R, kind="ExternalInput")
    wv = nc.dram_tensor("wv", [D, CW], F32R, kind="ExternalInput")
    bq = nc.dram_tensor("bq", [CW], F32, kind="ExternalInput")   # pre-scaled /8
    bk = nc.dram_tensor("bk", [CW], F32, kind="ExternalInput")
    wp = nc.dram_tensor("wp", [CW, D], F32R, kind="ExternalInput")
    mask = nc.dram_tensor("mask", [128, NQC, TCH], BF16, kind="ExternalInput")
    out = nc.dram_tensor("out", [B, T, D], BF16, kind="ExternalOutput")

    with tile.TileContext(nc) as tc:
        with (
            tc.tile_pool(name="const", bufs=1) as const,
            tc.tile_pool(name="persist", bufs=1) as persist,
            tc.tile_pool(name="xt", bufs=4) as xt_pool,
            tc.tile_pool(name="probs", bufs=6) as probs_pool,
            tc.tile_pool(name="norm", bufs=3) as norm_pool,
            tc.tile_pool(name="osb", bufs=3) as osb_pool,
        ):
            # ---- constants / persistent state ----
            # issue-order matters at kernel start: the first QKV matmuls need
            # wq/wk and the first x d-tiles; everything else can trail.
            wq_sb = const.tile([128, D // 128, CW], F32R)
            wk_sb = const.tile([128, D // 128, CW], F32R)
            wv_sb = const.tile([128, D // 128, CW], F32R)
            wp_sb = const.tile([128, D], F32R)
            bq_sb = const.tile([128, 1], F32)
            bk_sb = const.tile([128, 1], F32)
            mask_sb = const.tile([128, NQC, TCH], BF16)
            # consts go on the gpsimd DMA queue so the first x chunks (sync
            # queue) land in parallel
            wq_r = wq.rearrange("(dt p) m -> p dt m", p=128)
            wk_r = wk.rearrange("(dt p) m -> p dt m", p=128)
            nc.gpsimd.dma_start(wq_sb[:, 0:1, :], wq_r[:, 0:1, :])
            nc.gpsimd.dma_start(wk_sb[:, 0:1, :], wk_r[:, 0:1, :])
            nc.gpsimd.dma_start(wq_sb[:, 1:, :], wq_r[:, 1:, :])
            nc.gpsimd.dma_start(wk_sb[:, 1:, :], wk_r[:, 1:, :])
            nc.gpsimd.dma_start(wv_sb[:], wv.rearrange("(dt p) m -> p dt m", p=128))
            nc.gpsimd.dma_start(bq_sb[:], bq[:, None])
            nc.gpsimd.dma_start(bk_sb[:], bk[:, None])
            nc.gpsimd.dma_start(wp_sb[:], wp[:, :])
            wp2_sb = const.tile([HD, HPC, D], F32R)
            nc.gpsimd.dma_start(wp2_sb[:], wp.rearrange("(h d) e -> d h e", h=HPC))
            nc.gpsimd.dma_start(mask_sb[:], mask[:, :, :])

            qT_sb = persist.tile([128, B * T], F32R)   # [2h*64, (b,t)]
            kT_sb = persist.tile([128, B * T], F32R)
            a_sb = persist.tile([128, B * T], F32R)    # normalized attn out ^T
            # v per t-tile & head: [v(64) | ones] columns — the trailing ones
            # column makes the AV matmul emit the softmax denominator in
            # psum partition 64 (64-aligned, so DVE ops can touch it).
            v_sb = persist.tile([128, B * T // 128, HPC, HD + 1], BF16)
            nc.vector.memset(v_sb[:, :, :, HD], 1.0)

            def body(_=None):
                # All psum pools coexist (8 banks total) so phase 2 for batch
                # 0 can start while phase 1 still processes batch 1's chunks.
                with (
                    tc.tile_pool(name="ps_qk", bufs=2, space="PSUM") as ps_qk,
                    tc.tile_pool(name="ps_v", bufs=1, space="PSUM") as ps_v,
                    tc.tile_pool(name="ps_s", bufs=2, space="PSUM") as ps_s,
                    tc.tile_pool(name="ps_av", bufs=2, space="PSUM") as ps_av,
                    tc.tile_pool(name="ps_o", bufs=1, space="PSUM") as ps_o,
                ):
                    # ============= phase 1: QKV projections ==================
                    for c in range(NTCH):
                        xt = xt_pool.tile([128, D // 128, TCH], F32R)
                        b_i, qc = divmod(c, NQC)
                        xsrc = xT[b_i].rearrange("(dt p) t -> p dt t", p=128)[
                            :, :, ds(qc * TCH, TCH)]
                        # split per pair of d-tiles so the first matmul can
                        # start as soon as its slice lands (first chunk:
                        # per-d-tile for the fastest possible PE start)
                        gw = 1 if c == 0 else 2
                        for g in range((D // 128) // gw):
                            nc.sync.dma_start(xt[:, ts(g, gw), :],
                                              xsrc[:, ts(g, gw), :])
                        q_ps = ps_qk.tile([128, TCH], F32, tag="qk")
                        k_ps = ps_qk.tile([128, TCH], F32, tag="qk")
                        v_ps = ps_v.tile([128, TCH], F32)
                        # q/k interleaved per d-tile so matmuls start as
                        # DMA slices land; v after (needs the full chunk)
                        nd = D // 128
                        for dt in range(nd):
                            nc.tensor.matmul(q_ps[:], wq_sb[:, dt, :],
                                             xt[:, dt, :], start=dt == 0,
                                             stop=dt == nd - 1)
                            nc.tensor.matmul(k_ps[:], wk_sb[:, dt, :],
                                             xt[:, dt, :], start=dt == 0,
                                             stop=dt == nd - 1)
                        for s in range(TCH // 128):
                            for dt in range(nd):
                                nc.tensor.matmul(
                                    v_ps[:, ts(s, 128)],
                                    xt[:, dt, ts(s, 128)],
                                    wv_sb[:, dt, :],
                                    start=(dt == 0), stop=(dt == nd - 1))
                        # epilogues: q = psum/8 + bq/8 ; k = psum + bk
                        nc.vector.tensor_scalar(
                            qT_sb[:, ds(c * TCH, TCH)], q_ps[:], 0.125,
                            bq_sb[:],
                            mybir.AluOpType.mult, mybir.AluOpType.add)
                        nc.vector.tensor_scalar_add(
                            kT_sb[:, ds(c * TCH, TCH)], k_ps[:], bk_sb[:])
                        for s in range(TCH // 128):
                            tt = c * (TCH // 128) + s
                            nc.any.tensor_copy(
                                v_sb[:, tt, :, 0:HD],
                                v_ps[:, ds(s * 128, 128)].rearrange(
                                    "p (h d) -> p h d", h=HPC))

                    if "2" not in phases:
                        return
                    # ===== phases 2+3: attention + projection ================
                    # both heads per (batch, q-chunk): their 64-contraction
                    # score matmuls sit on partition bases 0/64 so the PE runs
                    # them concurrently (row groups)
                    for b_i in range(B):
                        for qc in range(NQC):
                            qcol = ds(b_i * T + qc * TCH, TCH)
                            nkt = 4 * qc + 4   # causal: k-tiles 0..nkt-1
                            avs = [ps_av.tile([HD + 1, TCH], F32, tag="av",
                                              name=f"av{_h}")
                                   for _h in range(HPC)]
                            for kt in range(nkt):
                                j = kt - 4 * qc   # >=0: straddles diagonal
                                # columns f < 128j are fully masked — skip
                                f0 = max(j, 0) * 128
                                fsl = ds(f0, TCH - f0)
                                ktcol = ds(b_i * T + kt * 128, 128)
                                sps = [ps_s.tile([128, TCH], F32, tag="s",
                                                 name=f"s{_h}")
                                       for _h in range(HPC)]
                                for h in range(HPC):
                                    hp = ds(h * HD, HD)
                                    nc.tensor.matmul(
                                        sps[h][:, fsl],
                                        kT_sb[hp, ktcol],
                                        qT_sb[hp, qcol][:, fsl],
                                        start=True, stop=True)
                                pp_sb = probs_pool.tile([128, HPC, TCH], BF16)
                                for h in range(HPC):
                                    nc.scalar.activation(
                                        pp_sb[:, h, fsl], sps[h][:, fsl],
                                        mybir.ActivationFunctionType.Exp)
                                if j >= 0:
                                    for h in range(HPC):
                                        nc.vector.tensor_tensor(
                                            pp_sb[:, h, fsl], pp_sb[:, h, fsl],
                                            mask_sb[:, j, fsl],
                                            mybir.AluOpType.mult)
                                for h in range(HPC):
                                    nc.tensor.matmul(
                                        avs[h][:, fsl],
                                        v_sb[:, b_i * NKT + kt, h, :],
                                        pp_sb[:, h, fsl],
                                        start=(kt == 0), stop=(kt == nkt - 1))
                            # normalize rows 0..63 by the ones-row 64
                            last = (b_i == B - 1 and qc == NQC - 1)
                            at_tiles = []
                            for h in range(HPC):
                                hp = ds(h * HD, HD)
                                av_ps = avs[h]
                                r64_sb = norm_pool.tile([HD + 1, TCH], F32,
                                                        tag="r64")
                                nc.vector.reciprocal(
                                    r64_sb[HD:HD + 1, :], av_ps[HD:HD + 1, :])
                                # partition_broadcast needs its source at
                                # physical partition 0 — DMA-shift it down
                                r0_sb = norm_pool.tile([1, TCH], F32, tag="r0")
                                nc.sync.dma_start(r0_sb[:],
                                                  r64_sb[HD:HD + 1, :])
                                bc_sb = norm_pool.tile([HD, TCH], F32,
                                                       tag="bc")
                                nc.gpsimd.partition_broadcast(
                                    bc_sb[:], r0_sb[:])
                                at_sb = norm_pool.tile([HD, TCH], F32R,
                                                       tag="at")
                                nc.vector.tensor_tensor(
                                    at_sb[:], av_ps[0:HD, :], bc_sb[:],
                                    mybir.AluOpType.mult)
                                at_tiles.append(at_sb)
                                if not last:
                                    # partition-shift into stacked-head layout
                                    nc.sync.dma_start(a_sb[hp, qcol],
                                                      at_sb[:])

                            if "3" not in phases:
                                continue
                            # projection for this q-chunk — interleaves with
                            # the next chunk's attention. The very last chunk
                            # has nothing to hide behind, so it reads the
                            # per-head at tiles directly (starts as soon as
                            # each head's normalize lands) and borrows av-tag
                            # psum slots to pipeline the copies.
                            for i, (tt, ec) in enumerate(
                                    (t_, e_)
                                    for t_ in range(qc * 4, qc * 4 + 4)
                                    for e_ in range(D // TCH)):
                                o_ps = ps_o.tile([128, TCH], F32)
                                if last:
                                    tloc = ds((tt - qc * 4) * 128, 128)
                                    for h in range(HPC):
                                        nc.tensor.matmul(
                                            o_ps[:],
                                            at_tiles[h][:, tloc],
                                            wp2_sb[:, h, ts(ec, TCH)],
                                            start=(h == 0), stop=(h == HPC - 1))
                                else:
                                    nc.tensor.matmul(
                                        o_ps[:],
                                        a_sb[:, ds(b_i * T + tt * 128, 128)],
                                        wp_sb[:, ts(ec, TCH)],
                                        start=True, stop=True)
                                o_sb = osb_pool.tile([128, TCH], BF16)
                                nc.any.tensor_copy(o_sb[:], o_ps[:])
                                nc.sync.dma_start(
                                    out[b_i, ts(tt, 128), ts(ec, TCH)],
                                    o_sb[:])

            if reps == 1:
                body()
            else:
                with tc.For_i(0, reps, 1) as _i:
                    body(_i)

    nc.compile()
    return nc


def make_mask() -> np.ndarray:
    """Multiplicative mask strips for the 4 diagonal-straddling k-tiles of a
    512-wide q-chunk: strip j keeps (p <= f - 128*j)."""
    p = np.arange(128)[:, None]
    f = np.arange(TCH)[None, :]
    m = np.stack([(p <= f - 128 * j) for j in range(NQC)], axis=1)
    import ml_dtypes
    return m.astype(ml_dtypes.bfloat16)


def make_in_maps(x, w_attn, b_attn, w_proj):
    xT = np.ascontiguousarray(np.transpose(x, (0, 2, 1)), dtype=np.float32)
    mask = make_mask()
    in_maps = []
    for c in range(N_CORES):
        cs = slice(CW * c, CW * (c + 1))
        in_maps.append({
            "xT": xT,
            "wq": np.ascontiguousarray(w_attn[:, 0 * D:1 * D][:, cs]),
            "wk": np.ascontiguousarray(w_attn[:, 1 * D:2 * D][:, cs]),
            "wv": np.ascontiguousarray(w_attn[:, 2 * D:3 * D][:, cs]),
            "bq": np.ascontiguousarray(b_attn[0 * D:1 * D][cs]) * 0.125,
            "bk": np.ascontiguousarray(b_attn[1 * D:2 * D][cs]),
            "wp": np.ascontiguousarray(w_proj[cs, :]),
            "mask": mask,
        })
    return in_maps


def host_bias(b_attn, b_proj, w_proj):
    # v-bias propagates exactly through softmax (rows sum to 1) and the linear
    # projection: out += b_v @ w_proj + b_proj
    return b_proj.astype(np.float32) + b_attn[2 * D:3 * D].astype(np.float32) @ w_proj.astype(np.float32)


_NC_CACHE = {}


def get_program(reps: int = 1, phases: str = "123"):
    key = (reps, phases)
    if key not in _NC_CACHE:
        _NC_CACHE[key] = build_program(reps, phases)
    return _NC_CACHE[key]


def kernel(x, w_attn, b_attn, w_proj, b_proj):
    x = np.asarray(x, np.float32)
    w_attn = np.asarray(w_attn, np.float32)
    b_attn = np.asarray(b_attn, np.float32)
    w_proj = np.asarray(w_proj, np.float32)
    b_proj = np.asarray(b_proj, np.float32)

    nc = get_program()
    in_maps = make_in_maps(x, w_attn, b_attn, w_proj)
    res = run_bass_kernel_spmd(nc, in_maps, core_ids=list(range(N_CORES)))
    acc = np.zeros((B, T, D), np.float64)
    for r in res.results:
        acc += r["out"].astype(np.float64)
    acc += host_bias(b_attn, b_proj, w_proj).astype(np.float64)
    return acc.astype(np.float32)
